# revision 1
# baseline (speedup 1.0000x reference)
"""Trainium2 Bass kernel for a 12-head attention layer with RoPE + causal SDPA.

Problem shapes (hardcoded): B=4, S=2048, E=1152, H=12, D=96.

Sharding: 8 cores = 4 batches x 2 head-groups (6 heads each). Each core:
  - transposes its batch's logits to X^T on-chip (PE transpose)
  - computes Q^T, K^T (per-head, transposed layout) and V (natural layout,
    with a fused all-ones column per head for softmax denominators)
  - applies RoPE (head-dim permutation folded into Wq/Wk host-side so the
    interleaved-pair rotation becomes rotate-half form)
  - causal SDPA with scores in S^T [k, q] layout (softmax normalization via
    denominator row + gpsimd partition-broadcast at the ctx stage)
  - partial output projection over its 6 heads, interleaved per q-chunk
Host sums the two partials per batch and adds bo.

Dtypes: float32r (TF32-like) for projections & output proj, bf16 for
scores/probs/attn-V, fp32 accumulation in PSUM.
"""
import sys

sys.path.insert(0, "/opt/trn_rl_repo")

import numpy as np
import ml_dtypes
from contextlib import ExitStack

import concourse.bass as bass
import concourse.tile as tile
from concourse import bacc, mybir
from concourse.bass_utils import run_bass_kernel_spmd
from concourse.masks import make_identity

F32 = mybir.dt.float32
F32R = mybir.dt.float32r
BF16 = mybir.dt.bfloat16

B, S, E, H, D = 4, 2048, 1152, 12, 96
H6 = 6                    # heads per core
KC = E // 128             # 9 contraction chunks
TT = S // 128             # 16 token tiles
QCW = 512                 # query-chunk width
NQC = S // QCW            # 4 query chunks
SCALE = 1.0 / float(np.sqrt(D))
ROPE_BASE = 10000.0


def _emit(nc, tc, t, rep):
    """Emit one full kernel body. t = dict of DRAM APs."""
    with ExitStack() as top:
        persist = top.enter_context(tc.tile_pool(name=f"persist{rep}", bufs=1))
        qk_bf = {}
        for h in range(H6):
            qk_bf[("q", h)] = persist.tile([D, S], BF16, tag=f"qbf{h}", name=f"qbf{h}")
            qk_bf[("k", h)] = persist.tile([D, S], BF16, tag=f"kbf{h}", name=f"kbf{h}")
        v_sb = [persist.tile([128, H6 * 97], BF16, tag=f"v{i}", name=f"v{i}")
                for i in range(TT)]

        ident = persist.tile([128, 128], F32, tag="ident")
        make_identity(nc, ident[:])
        ones_f = persist.tile([1, 128], F32, tag="ones_f")
        nc.vector.memset(ones_f[:], 1.0)
        ones_r = persist.tile([1, 128], F32R, tag="ones_r")
        nc.vector.tensor_copy(ones_r[:], ones_f[:])

        bq_sb = persist.tile([D, H6], F32, tag="bq")
        nc.sync.dma_start(bq_sb[:], t["bq"])
        bk_sb = persist.tile([D, H6], F32, tag="bk")
        nc.sync.dma_start(bk_sb[:], t["bk"])

        # ================= Phase A: X^T, projections, RoPE =================
        with tc.tile_pool(name=f"pa{rep}", bufs=1) as pa, \
             tc.tile_pool(name=f"pa_ps{rep}", bufs=1, space="PSUM") as pa_ps:
            cos_sb = pa.tile([D, S], F32, tag="cos")
            nc.sync.dma_start(cos_sb[:], t["cosext"])
            sin_sb = pa.tile([D, S], F32, tag="sin")
            nc.sync.dma_start(sin_sb[:], t["sinext"])

            for hf in range(2):
                hw = S // 2
                t0 = hf * (TT // 2)
                # -- X^T for this half --
                xT = [pa.tile([128, hw], F32R, tag=f"xT{k}", name=f"xT{k}")
                      for k in range(KC)]
                for tl in range(TT // 2):
                    tt = t0 + tl
                    xn = pa.tile([128, E], F32, tag="xnat", bufs=2)
                    nc.sync.dma_start(xn[:], t["x"][tt * 128:(tt + 1) * 128, :])
                    for kg in range(3):
                        kcs = range(kg * 4, min(KC, kg * 4 + 4))
                        tp = pa_ps.tile([128, 128 * len(kcs)], F32, tag="trans", bufs=2)
                        for i, k in enumerate(kcs):
                            nc.tensor.transpose(
                                tp[:, i * 128:(i + 1) * 128],
                                xn[:, k * 128:(k + 1) * 128], ident[:])
                        for i, k in enumerate(kcs):
                            eng = nc.scalar.copy if k % 2 == 0 else nc.vector.tensor_copy
                            eng(xT[k][:, tl * 128:(tl + 1) * 128],
                                tp[:, i * 128:(i + 1) * 128])

                # -- V projection (natural layout, fused bias+ones row) --
                wv_sb = [pa.tile([128, H6 * 97], F32R, tag=f"wv{k}", name=f"wv{k}")
                         for k in range(KC)]
                for k in range(KC):
                    nc.sync.dma_start(wv_sb[k][:], t["wv"][k * 128:(k + 1) * 128, :])
                wv_b = pa.tile([1, H6 * 97], F32R, tag="wvb")
                nc.sync.dma_start(wv_b[:], t["wv"][E:E + 1, :])
                NB = (288, 294)  # psum bank split of 582 (f32r needs even N)
                for tl in range(TT // 2):
                    tt = t0 + tl
                    vps = [pa_ps.tile([128, n], F32, tag=f"vps{i}", bufs=2,
                                      name=f"vps{i}") for i, n in enumerate(NB)]
                    for k in range(KC + 1):
                        lhsT = (ones_r[:] if k == KC
                                else xT[k][:, tl * 128:(tl + 1) * 128])
                        c0 = 0
                        for i, n in enumerate(NB):
                            rhs = (wv_b[:, c0:c0 + n] if k == KC
                                   else wv_sb[k][:, c0:c0 + n])
                            nc.tensor.matmul(vps[i][:], lhsT, rhs,
                                             start=(k == 0), stop=(k == KC))
                            c0 += n
                    c0 = 0
                    for i, n in enumerate(NB):
                        nc.scalar.copy(v_sb[tt][:, c0:c0 + n], vps[i][:])
                        c0 += n

                # -- Q^T / K^T projections + RoPE (head pairs share W tiles) --
                for hp in range(H6 // 2):
                    for which, wname, b_sb in (("q", "wq", bq_sb), ("k", "wk", bk_sb)):
                        wp = [pa.tile([128, 2 * D], F32R, tag=f"wp{k}", bufs=2,
                                      name=f"wp{k}") for k in range(KC)]
                        for k in range(KC):
                            nc.sync.dma_start(
                                wp[k][:],
                                t[wname][k * 128:(k + 1) * 128,
                                         hp * 2 * D:(hp + 1) * 2 * D])
                        for hl in range(2):
                            h = 2 * hp + hl
                            raw = pa.tile([D, hw], F32, tag="qkraw", bufs=2)
                            for qc in range(2):
                                ps = pa_ps.tile([D, QCW], F32, tag="qkps", bufs=2)
                                for k in range(KC):
                                    nc.tensor.matmul(
                                        ps[:], wp[k][:, hl * D:(hl + 1) * D],
                                        xT[k][:, qc * QCW:(qc + 1) * QCW],
                                        start=(k == 0), stop=(k == KC - 1))
                                nc.scalar.add(
                                    raw[:, qc * QCW:(qc + 1) * QCW], ps[:],
                                    b_sb[:, h:h + 1])
                            # RoPE (rotate-half form after host-side permutation)
                            oc = qk_bf[(which, h)][:, hf * hw:(hf + 1) * hw]
                            swp = pa.tile([D, hw], F32, tag="swp", bufs=2)
                            nc.sync.dma_start(swp[0:48, :], raw[48:96, :])
                            nc.sync.dma_start(swp[48:96, :], raw[0:48, :])
                            nc.vector.tensor_mul(
                                oc, raw[:], cos_sb[:, hf * hw:(hf + 1) * hw])
                            tmp = pa.tile([D, hw], BF16, tag="ropetmp", bufs=2)
                            nc.vector.tensor_mul(
                                tmp[:], swp[:], sin_sb[:, hf * hw:(hf + 1) * hw])
                            nc.vector.tensor_add(oc, oc, tmp[:])

        # ============ Phase B+C: causal SDPA + output projection ============
        with tc.tile_pool(name=f"pb{rep}", bufs=1) as pb, \
             tc.tile_pool(name=f"pb_ps{rep}", bufs=1, space="PSUM") as pb_ps:
            ctx_sb = [pb.tile([D, S], F32R, tag=f"ctx{h}", name=f"ctx{h}")
                      for h in range(H6)]
            msk_sb = pb.tile([128, 4 * QCW], BF16, tag="masks")
            nc.sync.dma_start(msk_sb[:], t["masks"])
            wo_sb = pb.tile([D, H6 * E], F32R, tag="wo")
            nc.sync.dma_start(wo_sb[:], t["wo"])
            NOB = 384

            for qc in range(NQC):
                nkc = 4 * qc + 4
                for h in range(H6):
                    qh, kh = qk_bf[("q", h)], qk_bf[("k", h)]
                    cps = pb_ps.tile([97, QCW], F32, tag="ctxps", bufs=2)
                    for kp in range(nkc // 2):
                        sps = pb_ps.tile([128, 2 * QCW], F32, tag="sps", bufs=2)
                        for i in range(2):
                            kc = 2 * kp + i
                            nc.tensor.matmul(
                                sps[:, i * QCW:(i + 1) * QCW],
                                kh[:, kc * 128:(kc + 1) * 128],
                                qh[:, qc * QCW:(qc + 1) * QCW],
                                start=True, stop=True)
                        pt = pb.tile([128, 2 * QCW], BF16, tag="pt", bufs=3)
                        nc.scalar.activation(pt[:], sps[:],
                                             mybir.ActivationFunctionType.Exp,
                                             scale=SCALE)
                        for i in range(2):
                            kc = 2 * kp + i
                            j = kc - 4 * qc
                            if j >= 0:  # diagonal-crossing chunk: causal mask
                                nc.vector.tensor_mul(
                                    pt[:, i * QCW:(i + 1) * QCW],
                                    pt[:, i * QCW:(i + 1) * QCW],
                                    msk_sb[:, j * QCW:(j + 1) * QCW])
                        for i in range(2):
                            kc = 2 * kp + i
                            nc.tensor.matmul(
                                cps[:], v_sb[kc][:, h * 97:(h + 1) * 97],
                                pt[:, i * QCW:(i + 1) * QCW],
                                start=(kc == 0), stop=(kc == nkc - 1))
                    # normalize: ctx[0:96] * (1 / ctx[96])
                    rec = pb.tile([1, QCW], F32, tag="rec", bufs=2)
                    with nc.allow_low_precision(reason="softmax reciprocal"):
                        nc.vector.reciprocal(rec[:], cps[96:97, :])
                    rsb = pb.tile([D, QCW], F32, tag="rsb", bufs=2)
                    nc.gpsimd.partition_broadcast(rsb[:], rec[:])
                    nc.vector.tensor_mul(
                        ctx_sb[h][:, qc * QCW:(qc + 1) * QCW], cps[0:96, :], rsb[:])

                # output projection for this q-chunk's token tiles
                for tl in range(4):
                    tt = 4 * qc + tl
                    osb = pb.tile([128, E], F32, tag="osb", bufs=2, name=f"osb{tt}")
                    for i in range(3):
                        ops = pb_ps.tile([128, NOB], F32, tag="ops", bufs=2)
                        for h in range(H6):
                            nc.tensor.matmul(
                                ops[:], ctx_sb[h][:, tt * 128:(tt + 1) * 128],
                                wo_sb[:, h * E + i * NOB:h * E + (i + 1) * NOB],
                                start=(h == 0), stop=(h == H6 - 1))
                        nc.vector.tensor_copy(osb[:, i * NOB:(i + 1) * NOB], ops[:])
                    nc.sync.dma_start(t["o"][tt * 128:(tt + 1) * 128, :], osb[:])


def build_nc(reps=1):
    nc = bacc.Bacc("TRN2", target_bir_lowering=False, debug=False, num_devices=8)
    t = {
        "x": nc.dram_tensor("x", [S, E], F32, kind="ExternalInput").ap(),
        "wq": nc.dram_tensor("wq", [E, H6 * D], F32R, kind="ExternalInput").ap(),
        "wk": nc.dram_tensor("wk", [E, H6 * D], F32R, kind="ExternalInput").ap(),
        "wv": nc.dram_tensor("wv", [E + 1, H6 * 97], F32R, kind="ExternalInput").ap(),
        "wo": nc.dram_tensor("wo", [D, H6 * E], F32R, kind="ExternalInput").ap(),
        "bq": nc.dram_tensor("bq", [D, H6], F32, kind="ExternalInput").ap(),
        "bk": nc.dram_tensor("bk", [D, H6], F32, kind="ExternalInput").ap(),
        "cosext": nc.dram_tensor("cosext", [D, S], F32, kind="ExternalInput").ap(),
        "sinext": nc.dram_tensor("sinext", [D, S], F32, kind="ExternalInput").ap(),
        "masks": nc.dram_tensor("masks", [128, 4 * QCW], BF16,
                                kind="ExternalInput").ap(),
        "o": nc.dram_tensor("o", [S, E], F32, kind="ExternalOutput").ap(),
    }
    with tile.TileContext(nc) as tc:
        for rep in range(reps):
            if rep:
                tc.strict_bb_all_engine_barrier()
            _emit(nc, tc, t, rep)
    nc.compile()
    return nc


_NC = None


def _get_nc():
    global _NC
    if _NC is None:
        _NC = build_nc()
    return _NC


def make_in_maps(logits, Wq, bq, Wk, bk, Wv, bv, Wo, bo):
    """Build the 8 per-core input maps (host-side sharding + preprocessing)."""
    logits = np.asarray(logits, np.float32)
    Wq, Wk, Wv, Wo = (np.asarray(a, np.float32) for a in (Wq, Wk, Wv, Wo))
    bq, bk, bv = (np.asarray(a, np.float32) for a in (bq, bk, bv))

    # head-dim permutation: interleaved pairs -> [even comps | odd comps]
    def perm_w(w):
        return (w.reshape(E, H, D // 2, 2).transpose(0, 1, 3, 2)
                .reshape(E, H * D))

    def perm_b(b):
        return b.reshape(H, D // 2, 2).transpose(0, 2, 1).reshape(H * D)

    wq_p, wk_p = perm_w(Wq), perm_w(Wk)
    bq_p, bk_p = perm_b(bq), perm_b(bk)

    # RoPE tables in [dim, token] layout, rotate-half form
    theta = (1.0 / ROPE_BASE ** (np.arange(0, D, 2, dtype=np.float64) / D))
    ang = np.arange(S, dtype=np.float64)[:, None] * theta[None, :]  # [S, 48]
    cos = np.cos(ang).T.astype(np.float32)  # [48, S]
    sin = np.sin(ang).T.astype(np.float32)
    cosext = np.vstack([cos, cos]).copy()
    sinext = np.vstack([-sin, sin]).copy()

    # causal masks for the 4 diagonal-crossing chunk offsets
    p = np.arange(128)[:, None]
    f = np.arange(QCW)[None, :]
    masks = np.concatenate(
        [(p <= f - 128 * j).astype(ml_dtypes.bfloat16) for j in range(4)], axis=1)

    in_maps = []
    for c in range(8):
        b_i = c // 2
        h0 = (c % 2) * H6
        cs, ce = h0 * D, (h0 + H6) * D

        wvp = np.zeros((E + 1, H6 * 97), np.float32)
        for hh in range(H6):
            g = (h0 + hh) * D
            wvp[:E, 97 * hh:97 * hh + D] = Wv[:, g:g + D]
            wvp[E, 97 * hh:97 * hh + D] = bv[g:g + D]
            wvp[E, 97 * hh + D] = 1.0

        wo_s = (Wo[cs:ce].reshape(H6, D, E).transpose(1, 0, 2)
                .reshape(D, H6 * E)).copy()

        in_maps.append({
            "x": np.ascontiguousarray(logits[b_i]),
            "wq": np.ascontiguousarray(wq_p[:, cs:ce]),
            "wk": np.ascontiguousarray(wk_p[:, cs:ce]),
            "wv": wvp,
            "wo": wo_s,
            "bq": np.ascontiguousarray(bq_p[cs:ce].reshape(H6, D).T),
            "bk": np.ascontiguousarray(bk_p[cs:ce].reshape(H6, D).T),
            "cosext": cosext,
            "sinext": sinext,
            "masks": masks,
        })
    return in_maps


def assemble_output(results, bo):
    bo = np.asarray(bo, np.float32)
    out = np.empty((B, S, E), np.float32)
    for b_i in range(B):
        out[b_i] = results[2 * b_i]["o"] + results[2 * b_i + 1]["o"] + bo
    return out


def kernel(logits, Wq, bq, Wk, bk, Wv, bv, Wo, bo, batch_size, seq_len):
    assert int(batch_size) == B and int(seq_len) == S
    nc = _get_nc()
    in_maps = make_in_maps(logits, Wq, bq, Wk, bk, Wv, bv, Wo, bo)
    res = run_bass_kernel_spmd(nc, in_maps, core_ids=list(range(8)))
    return assemble_output(res.results, bo)



# revision 3
# speedup vs baseline: 1.2021x; 1.2021x over previous
"""Trainium2 Bass kernel for a 12-head attention layer with RoPE + causal SDPA.

Problem shapes (hardcoded): B=4, S=2048, E=1152, H=12, D=96.

Sharding: 8 cores = 4 batches x 2 head-groups (6 heads each).

v2 redesign vs baseline:
  - X^T computed on host (free) -> no on-chip PE transposes / PSUM copies
  - all big operands (X^T, Wq, Wk, Wv, Wo, ctx) in bf16: half the DMA bytes,
    1024-wide moving operands for the QKV projections
  - batched weight/input DMAs (one DMA per tensor, chunk-major host layout)
    to cut DMA-issue sequencer cost; issue order puts first-needed first
  - ragged diagonal SDPA: causality exploited at 128-token granularity
    (136/256 blocks instead of 160/256), one [128,128] tril mask on gpsimd
  - RoPE in bf16 on DVE (2x mode); swap DMAs issued from the DVE queue
  - exp on ACT from PSUM in [128,<=1024] chunks

Host sums the two partials per batch and adds bo.
"""
import sys

sys.path.insert(0, "/opt/trn_rl_repo")

import numpy as np
import ml_dtypes
from contextlib import ExitStack

import concourse.bass as bass
import concourse.tile as tile
from concourse import bacc, mybir
from concourse.bass_utils import run_bass_kernel_spmd

F32 = mybir.dt.float32
F32R = mybir.dt.float32r
BF16 = mybir.dt.bfloat16

B, S, E, H, D = 4, 2048, 1152, 12, 96
H6 = 6                    # heads per core
KC = E // 128             # 9 contraction chunks
QCW = 512                 # query-chunk width (phase B)
NQC = S // QCW            # 4 query chunks
PW = 1024                 # projection moving width (bf16 allows 1024)
SCALE = 1.0 / float(np.sqrt(D))
ROPE_BASE = 10000.0
VW = H6 * 97              # 582: v columns incl. per-head denominator ones col


def _emit(nc, tc, t, rep):
    with ExitStack() as top:
        persist = top.enter_context(tc.tile_pool(name=f"persist{rep}", bufs=1))
        qk_bf = {}
        for h in range(H6):
            qk_bf[("q", h)] = persist.tile([D, S], BF16, tag=f"qbf{h}", name=f"qbf{h}")
            qk_bf[("k", h)] = persist.tile([D, S], BF16, tag=f"kbf{h}", name=f"kbf{h}")
        v_sb = [persist.tile([128, VW], BF16, tag=f"v{i}", name=f"v{i}")
                for i in range(S // 128)]

        ones_b = persist.tile([1, 128], BF16, tag="ones_b")
        nc.vector.memset(ones_b[:], 1.0)

        # ================= Phase A: QKV projections + RoPE =================
        # Pre-RoPE projections are written straight into qk_bf; RoPE is then
        # applied in place (oc *= cos; oc += swp*sin).
        with tc.tile_pool(name=f"pa{rep}", bufs=1) as pa, \
             tc.tile_pool(name=f"pa_ps{rep}", bufs=1, space="PSUM") as pa_ps:
            # DMA issue order on the SP queue = first-needed first. xT half 0
            # arrives interleaved with Wq chunks so the Q-projection warmup is
            # paced by DMA arrival from ~1us in. xT lives in the pa pool (it
            # is dead after the projections) so phase-B tiles alias its
            # early-freed space rather than blocking on the RoPE tail.
            xT = pa.tile([128, KC * S], BF16, tag="xt")
            wq_sb = persist.tile([128, KC * H6 * D], BF16, tag="wq")
            WQC = H6 * D
            for k in range(KC):
                nc.sync.dma_start(xT[:, k * S:k * S + 2 * QCW],
                                  t["xt"][:, k * S:k * S + 2 * QCW])
                nc.sync.dma_start(wq_sb[:, k * WQC:(k + 1) * WQC],
                                  t["wq"][:, k * WQC:(k + 1) * WQC])
            bq_sb = persist.tile([D, H6], F32, tag="bq")
            nc.sync.dma_start(bq_sb[:], t["bq"])
            bk_sb = persist.tile([D, H6], F32, tag="bk")
            nc.sync.dma_start(bk_sb[:], t["bk"])
            wv_sb = persist.tile([128, KC * VW], BF16, tag="wv")
            nc.sync.dma_start(wv_sb[:], t["wv"])
            wv_b = persist.tile([1, VW], BF16, tag="wvb")
            nc.sync.dma_start(wv_b[:], t["wvb"])
            wk_sb = persist.tile([128, KC * H6 * D], BF16, tag="wk")
            nc.sync.dma_start(wk_sb[:], t["wk"])
            for k in range(KC):
                nc.sync.dma_start(xT[:, k * S + 2 * QCW:(k + 1) * S],
                                  t["xt"][:, k * S + 2 * QCW:(k + 1) * S])
            cos_sb = persist.tile([D, S], BF16, tag="cos")
            nc.sync.dma_start(cos_sb[:], t["cosext"])
            sin_sb = persist.tile([D, S], BF16, tag="sin")
            nc.sync.dma_start(sin_sb[:], t["sinext"])
            wo_sb = persist.tile([D, H6 * E], BF16, tag="wo")
            nc.sync.dma_start(wo_sb[:], t["wo"])
            msk_sb = persist.tile([128, 128], BF16, tag="mask")
            nc.sync.dma_start(msk_sb[:], t["mask"])

            def qk_proj(which, wt, b_sb, h, q2, tag, bufs):
                # psum out is split 2x512: a f32 matmul output may not cross
                # a psum bank boundary
                ps = pa_ps.tile([D, PW], F32, tag=tag, bufs=bufs,
                                name=f"ps_{which}{h}_{q2}")
                for k in range(KC):
                    for hf in range(2):
                        nc.tensor.matmul(
                            ps[:, hf * QCW:(hf + 1) * QCW],
                            wt[:, k * H6 * D + h * D:k * H6 * D + (h + 1) * D],
                            xT[:, k * S + q2 * PW + hf * QCW:
                               k * S + q2 * PW + (hf + 1) * QCW],
                            start=(k == 0), stop=(k == KC - 1))
                nc.scalar.add(qk_bf[(which, h)][:, q2 * PW:(q2 + 1) * PW],
                              ps[:], b_sb[:, h:h + 1])

            def qk_proj_warm(which, wt, b_sb):
                # first 2 heads' q2=0 chains interleaved k-major so the PE is
                # fed by arriving DMA chunks with no long serial stall
                ps2 = [pa_ps.tile([D, PW], F32, tag="qkps", bufs=2,
                                  name=f"warm_{which}{h}") for h in range(2)]
                for k in range(KC):
                    for h in range(2):
                        for hf in range(2):
                            nc.tensor.matmul(
                                ps2[h][:, hf * QCW:(hf + 1) * QCW],
                                wt[:, k * H6 * D + h * D:k * H6 * D + (h + 1) * D],
                                xT[:, k * S + hf * QCW:k * S + (hf + 1) * QCW],
                                start=(k == 0), stop=(k == KC - 1))
                for h in range(2):
                    nc.scalar.add(qk_bf[(which, h)][:, 0:PW], ps2[h][:],
                                  b_sb[:, h:h + 1])

            def v_proj(tt):
                NB = (288, 294)  # psum bank split of 582
                vps = [pa_ps.tile([128, n], F32, tag=f"vps{i}", bufs=2,
                                  name=f"vps{i}_{tt}") for i, n in enumerate(NB)]
                for k in range(KC + 1):
                    lhsT = (ones_b[:] if k == KC
                            else xT[:, k * S + tt * 128:k * S + (tt + 1) * 128])
                    c0 = 0
                    for i, n in enumerate(NB):
                        rhs = (wv_b[:, c0:c0 + n] if k == KC
                               else wv_sb[:, k * VW + c0:k * VW + c0 + n])
                        nc.tensor.matmul(vps[i][:], lhsT, rhs,
                                         start=(k == 0), stop=(k == KC))
                        c0 += n
                c0 = 0
                for i, n in enumerate(NB):
                    nc.scalar.copy(v_sb[tt][:, c0:c0 + n], vps[i][:])
                    c0 += n

            # ordered to match DMA arrival: xt half0 + wq, then wv, wk, half1
            qk_proj_warm("q", wq_sb, bq_sb)
            for h in range(2, H6):
                qk_proj("q", wq_sb, bq_sb, h, 0, "qkps", 2)
            for tt in range(8):
                v_proj(tt)
            qk_proj_warm("k", wk_sb, bk_sb)
            for h in range(2, H6):
                qk_proj("k", wk_sb, bk_sb, h, 0, "qkps", 2)
            for tt in range(8, 16):
                v_proj(tt)
            # second token half + RoPE per head
            for h in range(H6):
                qk_proj("q", wq_sb, bq_sb, h, 1, "qkps", 2)
                qk_proj("k", wk_sb, bk_sb, h, 1, "qkps", 2)
                swp = persist.tile([D, 2 * S], BF16, tag="swpqk", bufs=2,
                                   name=f"swp{h}")
                nc.scalar.dma_start(swp[0:48, 0:S], qk_bf[("q", h)][48:96, :])
                nc.scalar.dma_start(swp[48:96, 0:S], qk_bf[("q", h)][0:48, :])
                nc.scalar.dma_start(swp[0:48, S:2 * S], qk_bf[("k", h)][48:96, :])
                nc.scalar.dma_start(swp[48:96, S:2 * S], qk_bf[("k", h)][0:48, :])
                for wi, which in enumerate(("q", "k")):
                    oc = qk_bf[(which, h)][:]
                    nc.vector.tensor_mul(oc, oc, cos_sb[:])
                    tmp = persist.tile([D, S], BF16, tag="ropetmp", bufs=2)
                    nc.vector.tensor_mul(tmp[:], swp[:, wi * S:(wi + 1) * S],
                                         sin_sb[:])
                    nc.vector.tensor_add(oc, oc, tmp[:])

        # ============ Phase B: causal SDPA + output projection ============
        with tc.tile_pool(name=f"pb{rep}", bufs=1) as pb, \
             tc.tile_pool(name=f"pb_ps{rep}", bufs=1, space="PSUM") as pb_ps:
            # ctx tiles are created lazily (first use is one head behind the
            # chunk pipeline) so the pt tag gets the lowest freed addresses —
            # the ones vacated by xT early in phase A — and phase-B exps never
            # wait on the RoPE tail.
            _ctx = {}

            def ctx_sb(h):
                if h not in _ctx:
                    _ctx[h] = pb.tile([D, S], BF16, tag=f"ctx{h}", name=f"ctx{h}")
                return _ctx[h]

            NOB = 384

            def normalize(h, qc, cps):
                # ctx[0:96] * (1 / ctx[96]); emitted one head late so the
                # Pool-queue broadcast never head-of-line-blocks mask muls
                rec = pb.tile([1, QCW], F32, tag="rec", bufs=2)
                with nc.allow_low_precision(reason="softmax reciprocal"):
                    nc.vector.reciprocal(rec[:], cps[96:97, :])
                rsb = pb.tile([D, QCW], F32, tag="rsb", bufs=2)
                nc.gpsimd.partition_broadcast(rsb[:], rec[:])
                nc.vector.tensor_mul(
                    ctx_sb(h)[:, qc * QCW:(qc + 1) * QCW], cps[0:96, :], rsb[:])

            def oproj(qc):
                # output projection for one q-chunk's token tiles
                for tl in range(4):
                    tt = 4 * qc + tl
                    osb = pb.tile([128, E], F32, tag="osb", bufs=2, name=f"osb{tt}")
                    for i in range(3):
                        ops = pb_ps.tile([128, NOB], F32, tag="ops", bufs=2)
                        for h in range(H6):
                            nc.tensor.matmul(
                                ops[:], ctx_sb(h)[:, tt * 128:(tt + 1) * 128],
                                wo_sb[:, h * E + i * NOB:h * E + (i + 1) * NOB],
                                start=(h == 0), stop=(h == H6 - 1))
                        nc.vector.tensor_copy(osb[:, i * NOB:(i + 1) * NOB], ops[:])
                    nc.sync.dma_start(t["o"][tt * 128:(tt + 1) * 128, :], osb[:])

            pending = None
            for qc in range(NQC):
                for h in range(H6):
                    qh, kh = qk_bf[("q", h)], qk_bf[("k", h)]
                    qcol = qc * QCW
                    cps = pb_ps.tile([97, QCW], F32, tag="cps", bufs=3)
                    # chunk list: full k-chunks then ragged diagonal ones
                    chunks = ([(kc, 0, QCW, False) for kc in range(4 * qc)] +
                              [(4 * qc + j, 128 * j, QCW - 128 * j, True)
                               for j in range(4)])
                    nch = len(chunks)

                    def scores(c):
                        kc, qoff, w, masked = c
                        sps = pb_ps.tile([128, QCW], F32, tag="sps", bufs=3)
                        nc.tensor.matmul(
                            sps[:, 0:w], kh[:, kc * 128:(kc + 1) * 128],
                            qh[:, qcol + qoff:qcol + QCW],
                            start=True, stop=True)
                        pt = pb.tile([128, QCW], BF16, tag="pt", bufs=4)
                        nc.scalar.activation(pt[:, 0:w], sps[:, 0:w],
                                             mybir.ActivationFunctionType.Exp,
                                             scale=SCALE)
                        if masked:
                            nc.gpsimd.tensor_mul(pt[:, 0:128], pt[:, 0:128],
                                                 msk_sb[:])
                        return pt

                    def att_v(c, pt, i):
                        kc, qoff, w, masked = c
                        nc.tensor.matmul(
                            cps[:, qoff:QCW], v_sb[kc][:, h * 97:(h + 1) * 97],
                            pt[:, 0:w], start=(i == 0), stop=(i == nch - 1))

                    # software pipeline: scores run 2 chunks ahead of att@V
                    pts = [scores(chunks[i]) for i in range(min(2, nch))]
                    for i in range(nch):
                        if i + 2 < nch:
                            pts.append(scores(chunks[i + 2]))
                        att_v(chunks[i], pts[i], i)
                    if pending is not None:
                        normalize(*pending)
                    pending = (h, qc, cps)
                    if h == 1 and qc > 0:
                        # previous q-chunk's output projection, emitted here so
                        # its wait on the last normalize overlaps this SDPA
                        oproj(qc - 1)
                if pending is not None:
                    normalize(*pending)
                    pending = None
            oproj(NQC - 1)


def build_nc(reps=1):
    nc = bacc.Bacc("TRN2", target_bir_lowering=False, debug=False, num_devices=8)
    t = {
        "xt": nc.dram_tensor("xt", [128, KC * S], BF16, kind="ExternalInput").ap(),
        "wq": nc.dram_tensor("wq", [128, KC * H6 * D], BF16,
                             kind="ExternalInput").ap(),
        "wk": nc.dram_tensor("wk", [128, KC * H6 * D], BF16,
                             kind="ExternalInput").ap(),
        "wv": nc.dram_tensor("wv", [128, KC * VW], BF16, kind="ExternalInput").ap(),
        "wvb": nc.dram_tensor("wvb", [1, VW], BF16, kind="ExternalInput").ap(),
        "wo": nc.dram_tensor("wo", [D, H6 * E], BF16, kind="ExternalInput").ap(),
        "bq": nc.dram_tensor("bq", [D, H6], F32, kind="ExternalInput").ap(),
        "bk": nc.dram_tensor("bk", [D, H6], F32, kind="ExternalInput").ap(),
        "cosext": nc.dram_tensor("cosext", [D, S], BF16, kind="ExternalInput").ap(),
        "sinext": nc.dram_tensor("sinext", [D, S], BF16, kind="ExternalInput").ap(),
        "mask": nc.dram_tensor("mask", [128, 128], BF16, kind="ExternalInput").ap(),
        "o": nc.dram_tensor("o", [S, E], F32, kind="ExternalOutput").ap(),
    }
    with tile.TileContext(nc) as tc:
        for rep in range(reps):
            if rep:
                tc.strict_bb_all_engine_barrier()
            _emit(nc, tc, t, rep)
    nc.compile()
    return nc


_NC = None


def _get_nc():
    global _NC
    if _NC is None:
        _NC = build_nc()
    return _NC


def _chunk_major(a, kc, rows):
    """[kc*rows, n] -> [rows, kc*n] with chunk-major columns."""
    n = a.shape[1]
    return np.ascontiguousarray(
        a.reshape(kc, rows, n).transpose(1, 0, 2).reshape(rows, kc * n))


def make_in_maps(logits, Wq, bq, Wk, bk, Wv, bv, Wo, bo):
    """Build the 8 per-core input maps (host-side sharding + preprocessing)."""
    BF = ml_dtypes.bfloat16
    logits = np.asarray(logits, np.float32)
    Wq, Wk, Wv, Wo = (np.asarray(a, np.float32) for a in (Wq, Wk, Wv, Wo))
    bq, bk, bv = (np.asarray(a, np.float32) for a in (bq, bk, bv))

    # head-dim permutation: interleaved pairs -> [even comps | odd comps]
    def perm_w(w):
        return (w.reshape(E, H, D // 2, 2).transpose(0, 1, 3, 2)
                .reshape(E, H * D))

    def perm_b(b):
        return b.reshape(H, D // 2, 2).transpose(0, 2, 1).reshape(H * D)

    wq_p, wk_p = perm_w(Wq), perm_w(Wk)
    bq_p, bk_p = perm_b(bq), perm_b(bk)

    # RoPE tables in [dim, token] layout, rotate-half form, bf16
    theta = (1.0 / ROPE_BASE ** (np.arange(0, D, 2, dtype=np.float64) / D))
    ang = np.arange(S, dtype=np.float64)[:, None] * theta[None, :]  # [S, 48]
    cos = np.cos(ang).T.astype(np.float32)  # [48, S]
    sin = np.sin(ang).T.astype(np.float32)
    cosext = np.vstack([cos, cos]).astype(BF)
    sinext = np.vstack([-sin, sin]).astype(BF)

    p = np.arange(128)[:, None]
    c = np.arange(128)[None, :]
    mask = (p <= c).astype(BF)

    in_maps = []
    for core in range(8):
        b_i = core // 2
        h0 = (core % 2) * H6
        cs, ce = h0 * D, (h0 + H6) * D

        xTb = np.ascontiguousarray(logits[b_i].T)  # [E, S]
        xt = _chunk_major(xTb, KC, 128).astype(BF)

        wvp = np.zeros((E, VW), np.float32)
        wvb = np.zeros((1, VW), np.float32)
        for hh in range(H6):
            g = (h0 + hh) * D
            wvp[:, 97 * hh:97 * hh + D] = Wv[:, g:g + D]
            wvb[0, 97 * hh:97 * hh + D] = bv[g:g + D]
            wvb[0, 97 * hh + D] = 1.0

        wo_s = (Wo[cs:ce].reshape(H6, D, E).transpose(1, 0, 2)
                .reshape(D, H6 * E)).astype(BF)

        in_maps.append({
            "xt": xt,
            "wq": _chunk_major(wq_p[:, cs:ce], KC, 128).astype(BF),
            "wk": _chunk_major(wk_p[:, cs:ce], KC, 128).astype(BF),
            "wv": _chunk_major(wvp, KC, 128).astype(BF),
            "wvb": wvb.astype(BF),
            "wo": wo_s,
            "bq": np.ascontiguousarray(bq_p[cs:ce].reshape(H6, D).T),
            "bk": np.ascontiguousarray(bk_p[cs:ce].reshape(H6, D).T),
            "cosext": cosext,
            "sinext": sinext,
            "mask": mask,
        })
    return in_maps


def assemble_output(results, bo):
    bo = np.asarray(bo, np.float32)
    out = np.empty((B, S, E), np.float32)
    for b_i in range(B):
        out[b_i] = results[2 * b_i]["o"] + results[2 * b_i + 1]["o"] + bo
    return out


def kernel(logits, Wq, bq, Wk, bk, Wv, bv, Wo, bo, batch_size, seq_len):
    assert int(batch_size) == B and int(seq_len) == S
    nc = _get_nc()
    in_maps = make_in_maps(logits, Wq, bq, Wk, bk, Wv, bv, Wo, bo)
    res = run_bass_kernel_spmd(nc, in_maps, core_ids=list(range(8)))
    return assemble_output(res.results, bo)


# revision 5
# speedup vs baseline: 1.3407x; 1.1153x over previous
"""Trainium2 Bass kernel for a 12-head attention layer with RoPE + causal SDPA.

Problem shapes (hardcoded): B=4, S=2048, E=1152, H=12, D=96.

Sharding: 8 cores = 4 batches x 2 head-groups (6 heads each).

v2 redesign vs baseline:
  - X^T computed on host (free) -> no on-chip PE transposes / PSUM copies
  - all big operands (X^T, Wq, Wk, Wv, Wo, ctx) in bf16: half the DMA bytes,
    1024-wide moving operands for the QKV projections
  - batched weight/input DMAs (one DMA per tensor, chunk-major host layout)
    to cut DMA-issue sequencer cost; issue order puts first-needed first
  - ragged diagonal SDPA: causality exploited at 128-token granularity
    (136/256 blocks instead of 160/256), one [128,128] tril mask on gpsimd
  - RoPE in bf16 on DVE (2x mode); swap DMAs issued from the DVE queue
  - exp on ACT from PSUM in [128,<=1024] chunks

Host sums the two partials per batch and adds bo.
"""
import sys

sys.path.insert(0, "/opt/trn_rl_repo")

import numpy as np
import ml_dtypes
from contextlib import ExitStack

import concourse.bass as bass
import concourse.tile as tile
from concourse import bacc, mybir
from concourse.bass_utils import run_bass_kernel_spmd

F32 = mybir.dt.float32
F32R = mybir.dt.float32r
BF16 = mybir.dt.bfloat16

B, S, E, H, D = 4, 2048, 1152, 12, 96
H6 = 6                    # heads per core
KC = E // 128             # 9 contraction chunks
QCW = 512                 # query-chunk width (phase B)
NQC = S // QCW            # 4 query chunks
PW = 1024                 # projection moving width (bf16 allows 1024)
SCALE = 1.0 / float(np.sqrt(D))
ROPE_BASE = 10000.0
VW = H6 * 97              # 582: v columns incl. per-head denominator ones col


def _emit(nc, tc, t, rep):
    with ExitStack() as top:
        persist = top.enter_context(tc.tile_pool(name=f"persist{rep}", bufs=1))
        qk_bf = {}
        for h in range(H6):
            qk_bf[("q", h)] = persist.tile([D, S], BF16, tag=f"qbf{h}", name=f"qbf{h}")
            qk_bf[("k", h)] = persist.tile([D, S], BF16, tag=f"kbf{h}", name=f"kbf{h}")
        v_sb = [persist.tile([128, VW], BF16, tag=f"v{i}", name=f"v{i}")
                for i in range(S // 128)]

        ones_b = persist.tile([1, 128], BF16, tag="ones_b")
        nc.vector.memset(ones_b[:], 1.0)

        # ================= Phase A: QKV projections + RoPE =================
        # Pre-RoPE projections are written straight into qk_bf; RoPE is then
        # applied in place (oc *= cos; oc += swp*sin).
        with tc.tile_pool(name=f"pa{rep}", bufs=1) as pa, \
             tc.tile_pool(name=f"pa_ps{rep}", bufs=1, space="PSUM") as pa_ps:
            # DMA issue order on the SP queue = first-needed first. xT half 0
            # arrives interleaved with Wq chunks so the Q-projection warmup is
            # paced by DMA arrival from ~1us in. xT lives in the pa pool (it
            # is dead after the projections) so phase-B tiles alias its
            # early-freed space rather than blocking on the RoPE tail.
            xT = pa.tile([128, KC * S], BF16, tag="xt")
            wq_sb = persist.tile([128, KC * H6 * D], BF16, tag="wq")
            WQC = H6 * D
            for k in range(KC):
                nc.sync.dma_start(xT[:, k * S:k * S + 2 * QCW],
                                  t["xt"][:, k * S:k * S + 2 * QCW])
                nc.sync.dma_start(wq_sb[:, k * WQC:(k + 1) * WQC],
                                  t["wq"][:, k * WQC:(k + 1) * WQC])
            bq_sb = persist.tile([D, H6], F32, tag="bq")
            nc.sync.dma_start(bq_sb[:], t["bq"])
            bk_sb = persist.tile([D, H6], F32, tag="bk")
            nc.sync.dma_start(bk_sb[:], t["bk"])
            wv_sb = persist.tile([128, KC * VW], BF16, tag="wv")
            nc.sync.dma_start(wv_sb[:], t["wv"])
            wv_b = persist.tile([1, VW], BF16, tag="wvb")
            nc.sync.dma_start(wv_b[:], t["wvb"])
            wk_sb = persist.tile([128, KC * H6 * D], BF16, tag="wk")
            nc.sync.dma_start(wk_sb[:], t["wk"])
            for k in range(KC):
                nc.sync.dma_start(xT[:, k * S + 2 * QCW:(k + 1) * S],
                                  t["xt"][:, k * S + 2 * QCW:(k + 1) * S])
            cos_sb = persist.tile([D, S], BF16, tag="cos")
            nc.sync.dma_start(cos_sb[:], t["cosext"])
            sin_sb = persist.tile([D, S], BF16, tag="sin")
            nc.sync.dma_start(sin_sb[:], t["sinext"])
            wo_sb = persist.tile([D, H6 * E], BF16, tag="wo")
            nc.sync.dma_start(wo_sb[:], t["wo"])
            msk_sb = persist.tile([128, 128], BF16, tag="mask")
            nc.sync.dma_start(msk_sb[:], t["mask"])

            def qk_proj(which, wt, b_sb, h, q2, tag, bufs):
                # psum out is split 2x512: a f32 matmul output may not cross
                # a psum bank boundary
                ps = pa_ps.tile([D, PW], F32, tag=tag, bufs=bufs,
                                name=f"ps_{which}{h}_{q2}")
                for k in range(KC):
                    for hf in range(2):
                        nc.tensor.matmul(
                            ps[:, hf * QCW:(hf + 1) * QCW],
                            wt[:, k * H6 * D + h * D:k * H6 * D + (h + 1) * D],
                            xT[:, k * S + q2 * PW + hf * QCW:
                               k * S + q2 * PW + (hf + 1) * QCW],
                            start=(k == 0), stop=(k == KC - 1))
                nc.scalar.add(qk_bf[(which, h)][:, q2 * PW:(q2 + 1) * PW],
                              ps[:], b_sb[:, h:h + 1])

            def qk_proj_warm(which, wt, b_sb):
                # first 2 heads' q2=0 chains interleaved k-major so the PE is
                # fed by arriving DMA chunks with no long serial stall
                ps2 = [pa_ps.tile([D, PW], F32, tag="qkps", bufs=2,
                                  name=f"warm_{which}{h}") for h in range(2)]
                for k in range(KC):
                    for h in range(2):
                        for hf in range(2):
                            nc.tensor.matmul(
                                ps2[h][:, hf * QCW:(hf + 1) * QCW],
                                wt[:, k * H6 * D + h * D:k * H6 * D + (h + 1) * D],
                                xT[:, k * S + hf * QCW:k * S + (hf + 1) * QCW],
                                start=(k == 0), stop=(k == KC - 1))
                for h in range(2):
                    nc.scalar.add(qk_bf[(which, h)][:, 0:PW], ps2[h][:],
                                  b_sb[:, h:h + 1])

            def v_proj(tt):
                NB = (288, 294)  # psum bank split of 582
                vps = [pa_ps.tile([128, n], F32, tag=f"vps{i}", bufs=2,
                                  name=f"vps{i}_{tt}") for i, n in enumerate(NB)]
                for k in range(KC + 1):
                    lhsT = (ones_b[:] if k == KC
                            else xT[:, k * S + tt * 128:k * S + (tt + 1) * 128])
                    c0 = 0
                    for i, n in enumerate(NB):
                        rhs = (wv_b[:, c0:c0 + n] if k == KC
                               else wv_sb[:, k * VW + c0:k * VW + c0 + n])
                        nc.tensor.matmul(vps[i][:], lhsT, rhs,
                                         start=(k == 0), stop=(k == KC))
                        c0 += n
                c0 = 0
                for i, n in enumerate(NB):
                    nc.scalar.copy(v_sb[tt][:, c0:c0 + n], vps[i][:])
                    c0 += n

            # ordered to match DMA arrival: xt half0 + wq, then wv, wk, half1
            qk_proj_warm("q", wq_sb, bq_sb)
            for h in range(2, H6):
                qk_proj("q", wq_sb, bq_sb, h, 0, "qkps", 2)
            for tt in range(8):
                v_proj(tt)
            qk_proj_warm("k", wk_sb, bk_sb)
            for h in range(2, H6):
                qk_proj("k", wk_sb, bk_sb, h, 0, "qkps", 2)
            for tt in range(8, 16):
                v_proj(tt)
            # second token half + RoPE per head
            for h in range(H6):
                qk_proj("q", wq_sb, bq_sb, h, 1, "qkps", 2)
                qk_proj("k", wk_sb, bk_sb, h, 1, "qkps", 2)
                swp = persist.tile([D, 2 * S], BF16, tag="swpqk", bufs=2,
                                   name=f"swp{h}")
                nc.scalar.dma_start(swp[0:48, 0:S], qk_bf[("q", h)][48:96, :])
                nc.scalar.dma_start(swp[48:96, 0:S], qk_bf[("q", h)][0:48, :])
                nc.scalar.dma_start(swp[0:48, S:2 * S], qk_bf[("k", h)][48:96, :])
                nc.scalar.dma_start(swp[48:96, S:2 * S], qk_bf[("k", h)][0:48, :])
                for wi, which in enumerate(("q", "k")):
                    oc = qk_bf[(which, h)][:]
                    nc.vector.tensor_mul(oc, oc, cos_sb[:])
                    tmp = persist.tile([D, S], BF16, tag="ropetmp", bufs=2)
                    nc.vector.tensor_mul(tmp[:], swp[:, wi * S:(wi + 1) * S],
                                         sin_sb[:])
                    nc.vector.tensor_add(oc, oc, tmp[:])

        # ============ Phase B: causal SDPA + output projection ============
        with tc.tile_pool(name=f"pb{rep}", bufs=1) as pb, \
             tc.tile_pool(name=f"pb_ps{rep}", bufs=1, space="PSUM") as pb_ps:
            # ctx tiles are created lazily (first use is one head behind the
            # chunk pipeline) so the pt tag gets the lowest freed addresses —
            # the ones vacated by xT early in phase A — and phase-B exps never
            # wait on the RoPE tail.
            _ctx = {}

            def ctx_sb(h):
                if h not in _ctx:
                    _ctx[h] = pb.tile([D, S], BF16, tag=f"ctx{h}", name=f"ctx{h}")
                return _ctx[h]

            NOB = 384

            def normalize(h, qc, cps):
                # ctx[0:96] * (1 / ctx[96]); emitted one head late so the
                # Pool-queue broadcast never head-of-line-blocks mask muls
                rec = pb.tile([1, QCW], F32, tag="rec", bufs=2)
                with nc.allow_low_precision(reason="softmax reciprocal"):
                    nc.vector.reciprocal(rec[:], cps[96:97, :])
                rsb = pb.tile([D, QCW], F32, tag="rsb", bufs=2)
                nc.gpsimd.partition_broadcast(rsb[:], rec[:])
                nc.vector.tensor_mul(
                    ctx_sb(h)[:, qc * QCW:(qc + 1) * QCW], cps[0:96, :], rsb[:])

            def oproj(qc):
                # output projection for one q-chunk's token tiles; ops shares
                # the spsr psum tag (same bank budget as the ragged scores)
                for tl in range(4):
                    tt = 4 * qc + tl
                    osb = pb.tile([128, E], F32, tag="osb", bufs=2, name=f"osb{tt}")
                    for i in range(3):
                        ops = pb_ps.tile([128, QCW], F32, tag="spsr", bufs=2)
                        for h in range(H6):
                            nc.tensor.matmul(
                                ops[:, 0:NOB],
                                ctx_sb(h)[:, tt * 128:(tt + 1) * 128],
                                wo_sb[:, h * E + i * NOB:h * E + (i + 1) * NOB],
                                start=(h == 0), stop=(h == H6 - 1))
                        nc.vector.tensor_copy(osb[:, i * NOB:(i + 1) * NOB],
                                              ops[:, 0:NOB])
                    nc.sync.dma_start(t["o"][tt * 128:(tt + 1) * 128, :], osb[:])

            pending = None
            for qc in range(NQC):
                for h in range(H6):
                    qh, kh = qk_bf[("q", h)], qk_bf[("k", h)]
                    qcol = qc * QCW
                    cps = pb_ps.tile([97, QCW], F32, tag="cps", bufs=2)
                    # units: full k-chunks in pairs (one [128,1024] exp each),
                    # then the ragged diagonal chunks singly
                    units = ([("pair", 2 * kp) for kp in range(2 * qc)] +
                             [("rag", 4 * qc + j) for j in range(4)])
                    nun = len(units)
                    nch = 4 * qc + 4

                    def scores(u):
                        kind, kc = u
                        if kind == "pair":
                            sps = pb_ps.tile([128, 2 * QCW], F32, tag="spsp",
                                             bufs=2)
                            for i in range(2):
                                nc.tensor.matmul(
                                    sps[:, i * QCW:(i + 1) * QCW],
                                    kh[:, (kc + i) * 128:(kc + i + 1) * 128],
                                    qh[:, qcol:qcol + QCW],
                                    start=True, stop=True)
                            pt = pb.tile([128, 2 * QCW], BF16, tag="ptp", bufs=3)
                            nc.scalar.activation(pt[:], sps[:],
                                                 mybir.ActivationFunctionType.Exp,
                                                 scale=SCALE)
                            return pt
                        j = kc - 4 * qc
                        w = QCW - 128 * j
                        sps = pb_ps.tile([128, QCW], F32, tag="spsr", bufs=2)
                        nc.tensor.matmul(
                            sps[:, 0:w], kh[:, kc * 128:(kc + 1) * 128],
                            qh[:, qcol + 128 * j:qcol + QCW],
                            start=True, stop=True)
                        pt = pb.tile([128, QCW], BF16, tag="ptr", bufs=3)
                        nc.scalar.activation(pt[:, 0:w], sps[:, 0:w],
                                             mybir.ActivationFunctionType.Exp,
                                             scale=SCALE)
                        nc.vector.tensor_mul(pt[:, 0:128], pt[:, 0:128],
                                             msk_sb[:])
                        return pt

                    def att_v(u, pt):
                        kind, kc = u
                        if kind == "pair":
                            for i in range(2):
                                nc.tensor.matmul(
                                    cps[:],
                                    v_sb[kc + i][:, h * 97:(h + 1) * 97],
                                    pt[:, i * QCW:(i + 1) * QCW],
                                    start=(kc + i == 0),
                                    stop=(kc + i == nch - 1))
                            return
                        j = kc - 4 * qc
                        w = QCW - 128 * j
                        nc.tensor.matmul(
                            cps[:, 128 * j:QCW], v_sb[kc][:, h * 97:(h + 1) * 97],
                            pt[:, 0:w], start=(kc == 0), stop=(kc == nch - 1))

                    # software pipeline: scores run 2 units ahead of att@V
                    pts = [scores(units[i]) for i in range(min(2, nun))]
                    for i in range(nun):
                        if i + 2 < nun:
                            pts.append(scores(units[i + 2]))
                        att_v(units[i], pts[i])
                    if pending is not None:
                        normalize(*pending)
                    pending = (h, qc, cps)
                    if h == 1 and qc > 0:
                        # previous q-chunk's output projection, emitted here so
                        # its wait on the last normalize overlaps this SDPA
                        oproj(qc - 1)
                if pending is not None:
                    normalize(*pending)
                    pending = None
            oproj(NQC - 1)


def build_nc(reps=1):
    nc = bacc.Bacc("TRN2", target_bir_lowering=False, debug=False, num_devices=8)
    t = {
        "xt": nc.dram_tensor("xt", [128, KC * S], BF16, kind="ExternalInput").ap(),
        "wq": nc.dram_tensor("wq", [128, KC * H6 * D], BF16,
                             kind="ExternalInput").ap(),
        "wk": nc.dram_tensor("wk", [128, KC * H6 * D], BF16,
                             kind="ExternalInput").ap(),
        "wv": nc.dram_tensor("wv", [128, KC * VW], BF16, kind="ExternalInput").ap(),
        "wvb": nc.dram_tensor("wvb", [1, VW], BF16, kind="ExternalInput").ap(),
        "wo": nc.dram_tensor("wo", [D, H6 * E], BF16, kind="ExternalInput").ap(),
        "bq": nc.dram_tensor("bq", [D, H6], F32, kind="ExternalInput").ap(),
        "bk": nc.dram_tensor("bk", [D, H6], F32, kind="ExternalInput").ap(),
        "cosext": nc.dram_tensor("cosext", [D, S], BF16, kind="ExternalInput").ap(),
        "sinext": nc.dram_tensor("sinext", [D, S], BF16, kind="ExternalInput").ap(),
        "mask": nc.dram_tensor("mask", [128, 128], BF16, kind="ExternalInput").ap(),
        "o": nc.dram_tensor("o", [S, E], F32, kind="ExternalOutput").ap(),
    }
    with tile.TileContext(nc) as tc:
        for rep in range(reps):
            if rep:
                tc.strict_bb_all_engine_barrier()
            _emit(nc, tc, t, rep)
    nc.compile()
    return nc


_NC = None


def _get_nc():
    global _NC
    if _NC is None:
        _NC = build_nc()
    return _NC


def _chunk_major(a, kc, rows):
    """[kc*rows, n] -> [rows, kc*n] with chunk-major columns."""
    n = a.shape[1]
    return np.ascontiguousarray(
        a.reshape(kc, rows, n).transpose(1, 0, 2).reshape(rows, kc * n))


def make_in_maps(logits, Wq, bq, Wk, bk, Wv, bv, Wo, bo):
    """Build the 8 per-core input maps (host-side sharding + preprocessing)."""
    BF = ml_dtypes.bfloat16
    logits = np.asarray(logits, np.float32)
    Wq, Wk, Wv, Wo = (np.asarray(a, np.float32) for a in (Wq, Wk, Wv, Wo))
    bq, bk, bv = (np.asarray(a, np.float32) for a in (bq, bk, bv))

    # head-dim permutation: interleaved pairs -> [even comps | odd comps]
    def perm_w(w):
        return (w.reshape(E, H, D // 2, 2).transpose(0, 1, 3, 2)
                .reshape(E, H * D))

    def perm_b(b):
        return b.reshape(H, D // 2, 2).transpose(0, 2, 1).reshape(H * D)

    wq_p, wk_p = perm_w(Wq), perm_w(Wk)
    bq_p, bk_p = perm_b(bq), perm_b(bk)

    # RoPE tables in [dim, token] layout, rotate-half form, bf16
    theta = (1.0 / ROPE_BASE ** (np.arange(0, D, 2, dtype=np.float64) / D))
    ang = np.arange(S, dtype=np.float64)[:, None] * theta[None, :]  # [S, 48]
    cos = np.cos(ang).T.astype(np.float32)  # [48, S]
    sin = np.sin(ang).T.astype(np.float32)
    cosext = np.vstack([cos, cos]).astype(BF)
    sinext = np.vstack([-sin, sin]).astype(BF)

    p = np.arange(128)[:, None]
    c = np.arange(128)[None, :]
    mask = (p <= c).astype(BF)

    in_maps = []
    for core in range(8):
        b_i = core // 2
        h0 = (core % 2) * H6
        cs, ce = h0 * D, (h0 + H6) * D

        xTb = np.ascontiguousarray(logits[b_i].T)  # [E, S]
        xt = _chunk_major(xTb, KC, 128).astype(BF)

        wvp = np.zeros((E, VW), np.float32)
        wvb = np.zeros((1, VW), np.float32)
        for hh in range(H6):
            g = (h0 + hh) * D
            wvp[:, 97 * hh:97 * hh + D] = Wv[:, g:g + D]
            wvb[0, 97 * hh:97 * hh + D] = bv[g:g + D]
            wvb[0, 97 * hh + D] = 1.0

        wo_s = (Wo[cs:ce].reshape(H6, D, E).transpose(1, 0, 2)
                .reshape(D, H6 * E)).astype(BF)

        in_maps.append({
            "xt": xt,
            "wq": _chunk_major(wq_p[:, cs:ce], KC, 128).astype(BF),
            "wk": _chunk_major(wk_p[:, cs:ce], KC, 128).astype(BF),
            "wv": _chunk_major(wvp, KC, 128).astype(BF),
            "wvb": wvb.astype(BF),
            "wo": wo_s,
            "bq": np.ascontiguousarray(bq_p[cs:ce].reshape(H6, D).T),
            "bk": np.ascontiguousarray(bk_p[cs:ce].reshape(H6, D).T),
            "cosext": cosext,
            "sinext": sinext,
            "mask": mask,
        })
    return in_maps


def assemble_output(results, bo):
    bo = np.asarray(bo, np.float32)
    out = np.empty((B, S, E), np.float32)
    for b_i in range(B):
        out[b_i] = results[2 * b_i]["o"] + results[2 * b_i + 1]["o"] + bo
    return out


def kernel(logits, Wq, bq, Wk, bk, Wv, bv, Wo, bo, batch_size, seq_len):
    assert int(batch_size) == B and int(seq_len) == S
    nc = _get_nc()
    in_maps = make_in_maps(logits, Wq, bq, Wk, bk, Wv, bv, Wo, bo)
    res = run_bass_kernel_spmd(nc, in_maps, core_ids=list(range(8)))
    return assemble_output(res.results, bo)


# revision 6
# speedup vs baseline: 1.4104x; 1.0520x over previous
"""Trainium2 Bass kernel for a 12-head attention layer with RoPE + causal SDPA.

Problem shapes (hardcoded): B=4, S=2048, E=1152, H=12, D=96.

Sharding: 8 cores = 4 batches x 2 head-groups (6 heads each).

v2 redesign vs baseline:
  - X^T computed on host (free) -> no on-chip PE transposes / PSUM copies
  - all big operands (X^T, Wq, Wk, Wv, Wo, ctx) in bf16: half the DMA bytes,
    1024-wide moving operands for the QKV projections
  - batched weight/input DMAs (one DMA per tensor, chunk-major host layout)
    to cut DMA-issue sequencer cost; issue order puts first-needed first
  - ragged diagonal SDPA: causality exploited at 128-token granularity
    (136/256 blocks instead of 160/256), one [128,128] tril mask on gpsimd
  - RoPE in bf16 on DVE (2x mode); swap DMAs issued from the DVE queue
  - exp on ACT from PSUM in [128,<=1024] chunks

Host sums the two partials per batch and adds bo.
"""
import sys

sys.path.insert(0, "/opt/trn_rl_repo")

import numpy as np
import ml_dtypes
from contextlib import ExitStack

import concourse.bass as bass
import concourse.tile as tile
from concourse import bacc, mybir
from concourse.bass_utils import run_bass_kernel_spmd

F32 = mybir.dt.float32
F32R = mybir.dt.float32r
BF16 = mybir.dt.bfloat16

B, S, E, H, D = 4, 2048, 1152, 12, 96
H6 = 6                    # heads per core
KC = E // 128             # 9 contraction chunks
QCW = 512                 # query-chunk width (phase B)
NQC = S // QCW            # 4 query chunks
PW = 1024                 # projection moving width (bf16 allows 1024)
SCALE = 1.0 / float(np.sqrt(D))
ROPE_BASE = 10000.0
VW = H6 * 97              # 582: v columns incl. per-head denominator ones col


def _emit(nc, tc, t, rep):
    with ExitStack() as top:
        persist = top.enter_context(tc.tile_pool(name=f"persist{rep}", bufs=1))
        qk_bf = {}
        for h in range(H6):
            qk_bf[("q", h)] = persist.tile([D, S], BF16, tag=f"qbf{h}", name=f"qbf{h}")
            qk_bf[("k", h)] = persist.tile([D, S], BF16, tag=f"kbf{h}", name=f"kbf{h}")
        v_sb = [persist.tile([128, VW], BF16, tag=f"v{i}", name=f"v{i}")
                for i in range(S // 128)]

        ones_b = persist.tile([1, 128], BF16, tag="ones_b")
        nc.vector.memset(ones_b[:], 1.0)

        # ================= Phase A: QKV projections + RoPE =================
        # Pre-RoPE projections are written straight into qk_bf; RoPE is then
        # applied in place (oc *= cos; oc += swp*sin).
        with tc.tile_pool(name=f"pa{rep}", bufs=1) as pa, \
             tc.tile_pool(name=f"pa_ps{rep}", bufs=1, space="PSUM") as pa_ps:
            # DMA issue order on the SP queue = first-needed first. xT half 0
            # arrives interleaved with Wq chunks so the Q-projection warmup is
            # paced by DMA arrival from ~1us in. xT lives in the pa pool (it
            # is dead after the projections) so phase-B tiles alias its
            # early-freed space rather than blocking on the RoPE tail.
            xT = pa.tile([128, KC * S], BF16, tag="xt")
            wq_sb = persist.tile([128, KC * H6 * D], BF16, tag="wq")
            WQC = H6 * D
            for k in range(KC):
                nc.sync.dma_start(xT[:, k * S:k * S + 2 * QCW],
                                  t["xt"][:, k * S:k * S + 2 * QCW])
                nc.sync.dma_start(wq_sb[:, k * WQC:(k + 1) * WQC],
                                  t["wq"][:, k * WQC:(k + 1) * WQC])
            bq_sb = persist.tile([D, H6], F32, tag="bq")
            nc.sync.dma_start(bq_sb[:], t["bq"])
            bk_sb = persist.tile([D, H6], F32, tag="bk")
            nc.sync.dma_start(bk_sb[:], t["bk"])
            wv_sb = persist.tile([128, KC * VW], BF16, tag="wv")
            nc.sync.dma_start(wv_sb[:], t["wv"])
            wv_b = persist.tile([1, VW], BF16, tag="wvb")
            nc.sync.dma_start(wv_b[:], t["wvb"])
            wk_sb = persist.tile([128, KC * H6 * D], BF16, tag="wk")
            nc.sync.dma_start(wk_sb[:], t["wk"])
            for k in range(KC):
                nc.sync.dma_start(xT[:, k * S + 2 * QCW:(k + 1) * S],
                                  t["xt"][:, k * S + 2 * QCW:(k + 1) * S])
            cos_sb = persist.tile([D, S], BF16, tag="cos")
            nc.sync.dma_start(cos_sb[:], t["cosext"])
            sin_sb = persist.tile([D, S], BF16, tag="sin")
            nc.sync.dma_start(sin_sb[:], t["sinext"])
            wo_sb = persist.tile([D, H6 * E], BF16, tag="wo")
            nc.sync.dma_start(wo_sb[:], t["wo"])
            msk01_sb = persist.tile([128, 896], BF16, tag="mask01")
            nc.sync.dma_start(msk01_sb[:], t["mask01"])
            msk23_sb = persist.tile([128, 384], BF16, tag="mask23")
            nc.sync.dma_start(msk23_sb[:], t["mask23"])

            def qk_proj(which, wt, b_sb, h, q2, tag, bufs):
                # psum out is split 2x512: a f32 matmul output may not cross
                # a psum bank boundary
                ps = pa_ps.tile([D, PW], F32, tag=tag, bufs=bufs,
                                name=f"ps_{which}{h}_{q2}")
                for k in range(KC):
                    for hf in range(2):
                        nc.tensor.matmul(
                            ps[:, hf * QCW:(hf + 1) * QCW],
                            wt[:, k * H6 * D + h * D:k * H6 * D + (h + 1) * D],
                            xT[:, k * S + q2 * PW + hf * QCW:
                               k * S + q2 * PW + (hf + 1) * QCW],
                            start=(k == 0), stop=(k == KC - 1))
                nc.scalar.add(qk_bf[(which, h)][:, q2 * PW:(q2 + 1) * PW],
                              ps[:], b_sb[:, h:h + 1])

            def qk_proj_warm(which, wt, b_sb):
                # first 2 heads' q2=0 chains interleaved k-major so the PE is
                # fed by arriving DMA chunks with no long serial stall
                ps2 = [pa_ps.tile([D, PW], F32, tag="qkps", bufs=2,
                                  name=f"warm_{which}{h}") for h in range(2)]
                for k in range(KC):
                    for h in range(2):
                        for hf in range(2):
                            nc.tensor.matmul(
                                ps2[h][:, hf * QCW:(hf + 1) * QCW],
                                wt[:, k * H6 * D + h * D:k * H6 * D + (h + 1) * D],
                                xT[:, k * S + hf * QCW:k * S + (hf + 1) * QCW],
                                start=(k == 0), stop=(k == KC - 1))
                for h in range(2):
                    nc.scalar.add(qk_bf[(which, h)][:, 0:PW], ps2[h][:],
                                  b_sb[:, h:h + 1])

            def v_proj(tt):
                NB = (288, 294)  # psum bank split of 582
                vps = [pa_ps.tile([128, n], F32, tag=f"vps{i}", bufs=2,
                                  name=f"vps{i}_{tt}") for i, n in enumerate(NB)]
                for k in range(KC + 1):
                    lhsT = (ones_b[:] if k == KC
                            else xT[:, k * S + tt * 128:k * S + (tt + 1) * 128])
                    c0 = 0
                    for i, n in enumerate(NB):
                        rhs = (wv_b[:, c0:c0 + n] if k == KC
                               else wv_sb[:, k * VW + c0:k * VW + c0 + n])
                        nc.tensor.matmul(vps[i][:], lhsT, rhs,
                                         start=(k == 0), stop=(k == KC))
                        c0 += n
                c0 = 0
                for i, n in enumerate(NB):
                    nc.scalar.copy(v_sb[tt][:, c0:c0 + n], vps[i][:])
                    c0 += n

            # ordered to match DMA arrival: xt half0 + wq, then wv, wk, half1
            qk_proj_warm("q", wq_sb, bq_sb)
            for h in range(2, H6):
                qk_proj("q", wq_sb, bq_sb, h, 0, "qkps", 2)
            for tt in range(8):
                v_proj(tt)
            qk_proj_warm("k", wk_sb, bk_sb)
            for h in range(2, H6):
                qk_proj("k", wk_sb, bk_sb, h, 0, "qkps", 2)
            for tt in range(8, 16):
                v_proj(tt)
            # second token half + RoPE per head
            for h in range(H6):
                qk_proj("q", wq_sb, bq_sb, h, 1, "qkps", 2)
                qk_proj("k", wk_sb, bk_sb, h, 1, "qkps", 2)
                swp = persist.tile([D, 2 * S], BF16, tag="swpqk", bufs=2,
                                   name=f"swp{h}")
                nc.scalar.dma_start(swp[0:48, 0:S], qk_bf[("q", h)][48:96, :])
                nc.scalar.dma_start(swp[48:96, 0:S], qk_bf[("q", h)][0:48, :])
                nc.scalar.dma_start(swp[0:48, S:2 * S], qk_bf[("k", h)][48:96, :])
                nc.scalar.dma_start(swp[48:96, S:2 * S], qk_bf[("k", h)][0:48, :])
                for wi, which in enumerate(("q", "k")):
                    oc = qk_bf[(which, h)][:]
                    nc.vector.tensor_mul(oc, oc, cos_sb[:])
                    tmp = persist.tile([D, S], BF16, tag="ropetmp", bufs=2)
                    nc.vector.tensor_mul(tmp[:], swp[:, wi * S:(wi + 1) * S],
                                         sin_sb[:])
                    nc.vector.tensor_add(oc, oc, tmp[:])

        # ============ Phase B: causal SDPA + output projection ============
        with tc.tile_pool(name=f"pb{rep}", bufs=1) as pb, \
             tc.tile_pool(name=f"pb_ps{rep}", bufs=1, space="PSUM") as pb_ps:
            # ctx tiles are created lazily (first use is one head behind the
            # chunk pipeline) so the pt tag gets the lowest freed addresses —
            # the ones vacated by xT early in phase A — and phase-B exps never
            # wait on the RoPE tail.
            _ctx = {}

            def ctx_sb(h):
                if h not in _ctx:
                    _ctx[h] = pb.tile([D, S], BF16, tag=f"ctx{h}", name=f"ctx{h}")
                return _ctx[h]

            NOB = 384

            def normalize(h, qc, cps):
                # ctx[0:96] * (1 / ctx[96]); emitted one head late so the
                # Pool-queue broadcast never head-of-line-blocks mask muls
                rec = pb.tile([1, QCW], F32, tag="rec", bufs=2)
                with nc.allow_low_precision(reason="softmax reciprocal"):
                    nc.vector.reciprocal(rec[:], cps[96:97, :])
                rsb = pb.tile([D, QCW], F32, tag="rsb", bufs=2)
                nc.gpsimd.partition_broadcast(rsb[:], rec[:])
                nc.vector.tensor_mul(
                    ctx_sb(h)[:, qc * QCW:(qc + 1) * QCW], cps[0:96, :], rsb[:])

            def oproj(qc):
                # output projection for one q-chunk's token tiles; ops shares
                # the spsr psum tag (same bank budget as the ragged scores)
                for tl in range(4):
                    tt = 4 * qc + tl
                    osb = pb.tile([128, E], F32, tag="osb", bufs=2, name=f"osb{tt}")
                    for i in range(3):
                        ops = pb_ps.tile([128, QCW], F32, tag="spsr", bufs=2)
                        for h in range(H6):
                            nc.tensor.matmul(
                                ops[:, 0:NOB],
                                ctx_sb(h)[:, tt * 128:(tt + 1) * 128],
                                wo_sb[:, h * E + i * NOB:h * E + (i + 1) * NOB],
                                start=(h == 0), stop=(h == H6 - 1))
                        nc.vector.tensor_copy(osb[:, i * NOB:(i + 1) * NOB],
                                              ops[:, 0:NOB])
                    nc.sync.dma_start(t["o"][tt * 128:(tt + 1) * 128, :], osb[:])

            pending = None
            for qc in range(NQC):
                for h in range(H6):
                    qh, kh = qk_bf[("q", h)], qk_bf[("k", h)]
                    qcol = qc * QCW
                    cps = pb_ps.tile([97, QCW], F32, tag="cps", bufs=2)
                    # units: full k-chunks in pairs (one [128,1024] exp each),
                    # then the ragged diagonal chunks packed (j0+j1 -> 896
                    # cols, j2+j3 -> 384 cols) with combined-triangle masks
                    units = ([("pair", 2 * kp) for kp in range(2 * qc)] +
                             [("rag01", 4 * qc), ("rag23", 4 * qc + 2)])
                    nun = len(units)
                    nch = 4 * qc + 4

                    def scores(u):
                        kind, kc = u
                        if kind == "pair":
                            sps = pb_ps.tile([128, 2 * QCW], F32, tag="spsp",
                                             bufs=2)
                            for i in range(2):
                                nc.tensor.matmul(
                                    sps[:, i * QCW:(i + 1) * QCW],
                                    kh[:, (kc + i) * 128:(kc + i + 1) * 128],
                                    qh[:, qcol:qcol + QCW],
                                    start=True, stop=True)
                            pt = pb.tile([128, 2 * QCW], BF16, tag="ptp", bufs=3)
                            nc.scalar.activation(pt[:], sps[:],
                                                 mybir.ActivationFunctionType.Exp,
                                                 scale=SCALE)
                            return pt
                        if kind == "rag01":
                            # chunk kc at cols [0:512] (q 0:512), kc+1 at
                            # [512:896] (q 128:512); diagonals at 0 and 512
                            sps = pb_ps.tile([128, 2 * QCW], F32, tag="spsp",
                                             bufs=2)
                            nc.tensor.matmul(
                                sps[:, 0:QCW], kh[:, kc * 128:(kc + 1) * 128],
                                qh[:, qcol:qcol + QCW], start=True, stop=True)
                            nc.tensor.matmul(
                                sps[:, QCW:QCW + 384],
                                kh[:, (kc + 1) * 128:(kc + 2) * 128],
                                qh[:, qcol + 128:qcol + QCW],
                                start=True, stop=True)
                            pt = pb.tile([128, 2 * QCW], BF16, tag="ptp", bufs=3)
                            nc.scalar.activation(pt[:, 0:896], sps[:, 0:896],
                                                 mybir.ActivationFunctionType.Exp,
                                                 scale=SCALE)
                            nc.vector.tensor_mul(pt[:, 0:896], pt[:, 0:896],
                                                 msk01_sb[:])
                            return pt
                        # rag23: chunk kc at [0:256] (q 256:512), kc+1 at
                        # [256:384] (q 384:512); diagonals at 0 and 256
                        sps = pb_ps.tile([128, QCW], F32, tag="spsr", bufs=2)
                        nc.tensor.matmul(
                            sps[:, 0:256], kh[:, kc * 128:(kc + 1) * 128],
                            qh[:, qcol + 256:qcol + QCW], start=True, stop=True)
                        nc.tensor.matmul(
                            sps[:, 256:384], kh[:, (kc + 1) * 128:(kc + 2) * 128],
                            qh[:, qcol + 384:qcol + QCW], start=True, stop=True)
                        pt = pb.tile([128, QCW], BF16, tag="ptr", bufs=3)
                        nc.scalar.activation(pt[:, 0:384], sps[:, 0:384],
                                             mybir.ActivationFunctionType.Exp,
                                             scale=SCALE)
                        nc.vector.tensor_mul(pt[:, 0:384], pt[:, 0:384],
                                             msk23_sb[:])
                        return pt

                    def att_v(u, pt):
                        kind, kc = u
                        if kind == "pair":
                            for i in range(2):
                                nc.tensor.matmul(
                                    cps[:],
                                    v_sb[kc + i][:, h * 97:(h + 1) * 97],
                                    pt[:, i * QCW:(i + 1) * QCW],
                                    start=(kc + i == 0),
                                    stop=(kc + i == nch - 1))
                            return
                        if kind == "rag01":
                            nc.tensor.matmul(
                                cps[:], v_sb[kc][:, h * 97:(h + 1) * 97],
                                pt[:, 0:QCW], start=(kc == 0), stop=False)
                            nc.tensor.matmul(
                                cps[:, 128:QCW],
                                v_sb[kc + 1][:, h * 97:(h + 1) * 97],
                                pt[:, QCW:QCW + 384], start=False, stop=False)
                            return
                        nc.tensor.matmul(
                            cps[:, 256:QCW], v_sb[kc][:, h * 97:(h + 1) * 97],
                            pt[:, 0:256], start=False, stop=False)
                        nc.tensor.matmul(
                            cps[:, 384:QCW], v_sb[kc + 1][:, h * 97:(h + 1) * 97],
                            pt[:, 256:384], start=False, stop=(kc + 1 == nch - 1))

                    # software pipeline: scores run 2 units ahead of att@V
                    pts = [scores(units[i]) for i in range(min(2, nun))]
                    for i in range(nun):
                        if i + 2 < nun:
                            pts.append(scores(units[i + 2]))
                        att_v(units[i], pts[i])
                    if pending is not None:
                        normalize(*pending)
                    pending = (h, qc, cps)
                    if h == 1 and qc > 0:
                        # previous q-chunk's output projection, emitted here so
                        # its wait on the last normalize overlaps this SDPA
                        oproj(qc - 1)
                if pending is not None:
                    normalize(*pending)
                    pending = None
            oproj(NQC - 1)


def build_nc(reps=1):
    nc = bacc.Bacc("TRN2", target_bir_lowering=False, debug=False, num_devices=8)
    t = {
        "xt": nc.dram_tensor("xt", [128, KC * S], BF16, kind="ExternalInput").ap(),
        "wq": nc.dram_tensor("wq", [128, KC * H6 * D], BF16,
                             kind="ExternalInput").ap(),
        "wk": nc.dram_tensor("wk", [128, KC * H6 * D], BF16,
                             kind="ExternalInput").ap(),
        "wv": nc.dram_tensor("wv", [128, KC * VW], BF16, kind="ExternalInput").ap(),
        "wvb": nc.dram_tensor("wvb", [1, VW], BF16, kind="ExternalInput").ap(),
        "wo": nc.dram_tensor("wo", [D, H6 * E], BF16, kind="ExternalInput").ap(),
        "bq": nc.dram_tensor("bq", [D, H6], F32, kind="ExternalInput").ap(),
        "bk": nc.dram_tensor("bk", [D, H6], F32, kind="ExternalInput").ap(),
        "cosext": nc.dram_tensor("cosext", [D, S], BF16, kind="ExternalInput").ap(),
        "sinext": nc.dram_tensor("sinext", [D, S], BF16, kind="ExternalInput").ap(),
        "mask01": nc.dram_tensor("mask01", [128, 896], BF16,
                                 kind="ExternalInput").ap(),
        "mask23": nc.dram_tensor("mask23", [128, 384], BF16,
                                 kind="ExternalInput").ap(),
        "o": nc.dram_tensor("o", [S, E], F32, kind="ExternalOutput").ap(),
    }
    with tile.TileContext(nc) as tc:
        for rep in range(reps):
            if rep:
                tc.strict_bb_all_engine_barrier()
            _emit(nc, tc, t, rep)
    nc.compile()
    return nc


_NC = None


def _get_nc():
    global _NC
    if _NC is None:
        _NC = build_nc()
    return _NC


def _chunk_major(a, kc, rows):
    """[kc*rows, n] -> [rows, kc*n] with chunk-major columns."""
    n = a.shape[1]
    return np.ascontiguousarray(
        a.reshape(kc, rows, n).transpose(1, 0, 2).reshape(rows, kc * n))


def make_in_maps(logits, Wq, bq, Wk, bk, Wv, bv, Wo, bo):
    """Build the 8 per-core input maps (host-side sharding + preprocessing)."""
    BF = ml_dtypes.bfloat16
    logits = np.asarray(logits, np.float32)
    Wq, Wk, Wv, Wo = (np.asarray(a, np.float32) for a in (Wq, Wk, Wv, Wo))
    bq, bk, bv = (np.asarray(a, np.float32) for a in (bq, bk, bv))

    # head-dim permutation: interleaved pairs -> [even comps | odd comps]
    def perm_w(w):
        return (w.reshape(E, H, D // 2, 2).transpose(0, 1, 3, 2)
                .reshape(E, H * D))

    def perm_b(b):
        return b.reshape(H, D // 2, 2).transpose(0, 2, 1).reshape(H * D)

    wq_p, wk_p = perm_w(Wq), perm_w(Wk)
    bq_p, bk_p = perm_b(bq), perm_b(bk)

    # RoPE tables in [dim, token] layout, rotate-half form, bf16
    theta = (1.0 / ROPE_BASE ** (np.arange(0, D, 2, dtype=np.float64) / D))
    ang = np.arange(S, dtype=np.float64)[:, None] * theta[None, :]  # [S, 48]
    cos = np.cos(ang).T.astype(np.float32)  # [48, S]
    sin = np.sin(ang).T.astype(np.float32)
    cosext = np.vstack([cos, cos]).astype(BF)
    sinext = np.vstack([-sin, sin]).astype(BF)

    p = np.arange(128)[:, None]
    c = np.arange(128)[None, :]
    tri = (p <= c)
    mask01 = np.ones((128, 896), np.float32)
    mask01[:, 0:128] = tri
    mask01[:, 512:640] = tri
    mask23 = np.ones((128, 384), np.float32)
    mask23[:, 0:128] = tri
    mask23[:, 256:384] = tri
    mask01 = mask01.astype(BF)
    mask23 = mask23.astype(BF)

    in_maps = []
    for core in range(8):
        b_i = core // 2
        h0 = (core % 2) * H6
        cs, ce = h0 * D, (h0 + H6) * D

        xTb = np.ascontiguousarray(logits[b_i].T)  # [E, S]
        xt = _chunk_major(xTb, KC, 128).astype(BF)

        wvp = np.zeros((E, VW), np.float32)
        wvb = np.zeros((1, VW), np.float32)
        for hh in range(H6):
            g = (h0 + hh) * D
            wvp[:, 97 * hh:97 * hh + D] = Wv[:, g:g + D]
            wvb[0, 97 * hh:97 * hh + D] = bv[g:g + D]
            wvb[0, 97 * hh + D] = 1.0

        wo_s = (Wo[cs:ce].reshape(H6, D, E).transpose(1, 0, 2)
                .reshape(D, H6 * E)).astype(BF)

        in_maps.append({
            "xt": xt,
            "wq": _chunk_major(wq_p[:, cs:ce], KC, 128).astype(BF),
            "wk": _chunk_major(wk_p[:, cs:ce], KC, 128).astype(BF),
            "wv": _chunk_major(wvp, KC, 128).astype(BF),
            "wvb": wvb.astype(BF),
            "wo": wo_s,
            "bq": np.ascontiguousarray(bq_p[cs:ce].reshape(H6, D).T),
            "bk": np.ascontiguousarray(bk_p[cs:ce].reshape(H6, D).T),
            "cosext": cosext,
            "sinext": sinext,
            "mask01": mask01,
            "mask23": mask23,
        })
    return in_maps


def assemble_output(results, bo):
    bo = np.asarray(bo, np.float32)
    out = np.empty((B, S, E), np.float32)
    for b_i in range(B):
        out[b_i] = results[2 * b_i]["o"] + results[2 * b_i + 1]["o"] + bo
    return out


def kernel(logits, Wq, bq, Wk, bk, Wv, bv, Wo, bo, batch_size, seq_len):
    assert int(batch_size) == B and int(seq_len) == S
    nc = _get_nc()
    in_maps = make_in_maps(logits, Wq, bq, Wk, bk, Wv, bv, Wo, bo)
    res = run_bass_kernel_spmd(nc, in_maps, core_ids=list(range(8)))
    return assemble_output(res.results, bo)
